# revision 20
# baseline (speedup 1.0000x reference)
"""NT-Xent (SimCLR) contrastive loss on 8 Trainium2 NeuronCores.

Math: with x_hat = row-normalized representation [8192, 256], tau = 0.5,
  sim = x_hat @ x_hat.T
  loss = (1/8192) * sum_i [ ln(sum_{j!=i} exp(2 sim[i,j])) - 2 sim[i, pos(i)] ]
where pos(i) = (i + 4096) mod 8192.

Sharding: data-parallel over rows; core c owns rows [c*1024, (c+1)*1024).
The host pre-normalizes rows, quantizes to fp8e4m3 (scaled by 4), and ships
each core a pre-transposed, row-rotated key matrix xT [128, 2, 8192] so a
single SPMD program works on every core: column j of core c's slab is global
row (j + c*1024) % 8192, which puts each core's own rows at columns 0..1023
(the matmul stationary tiles) and every core's positive diagonal at columns
4096..5119.

On device, per core: 128 fp8 DoubleRow matmuls (K=256 in one pass) build the
[1024, 8192] similarity slab in [128, 2048] PSUM chunks; the ACT engine does
exp (the scale folds the 1/16 fp8 scaling and 1/tau); row-sums ride the DVE
scalar_tensor_tensor accumulator except the last few chunks, which use the
ACT accumulator so the kernel doesn't end DVE-bound. The positive diagonal is
read lazily from the exp'd u=2 chunk (identity mask + reduce; the host takes
ln to recover 2*cos). Output is [128, 16] per core; the host finishes with
ln(S - e^2) - ln(pos_exp) summed over rows.
"""

import numpy as np
import ml_dtypes

import concourse.bacc as bacc
import concourse.bass as bass
import concourse.tile as tile
from concourse import mybir
from concourse.bass_utils import run_bass_kernel_spmd

N2 = 8192            # total rows (2N)
D = 256              # feature dim
NCORES = 8
ROWS = N2 // NCORES  # 1024 rows per core
N = N2 // 2          # positive-pair offset
P = 128              # SBUF partitions
KC = 2               # two 128-row contraction chunks (K=256 via DoubleRow)
T_SLAB = ROWS // P   # 8 row tiles of the slab
CW = 2048            # exp chunk width (4 PSUM banks)
NU = N2 // CW        # 4 chunks per slab row tile
MMW = 512            # matmul moving free width (1 PSUM bank)
FP8_SCALE = 4.0      # x_hat quantized as x_hat * 4 -> sim psum = 16*cos

F32 = mybir.dt.float32
BF16 = mybir.dt.bfloat16
FP8 = mybir.dt.float8e4
AF = mybir.ActivationFunctionType
ALU = mybir.AluOpType
DR = mybir.MatmulPerfMode.DoubleRow


def _build_kernel(tc: tile.TileContext, out_ap, xT_in, ident_in):
    nc = tc.nc
    with (
        tc.tile_pool(name="sb", bufs=1) as sb,
        tc.tile_pool(name="psmm", bufs=2, space="PSUM") as psmm,
    ):
        xT = sb.tile([P, KC, N2], FP8, name="xT")
        # stream the key matrix in column pieces so matmuls can start early;
        # the first piece is small to cut the pipeline-fill latency
        splits = [0, 512, 1024, 2048, 4096, N2]
        for lo, hi in zip(splits, splits[1:]):
            nc.sync.dma_start(out=xT[:, :, lo:hi], in_=xT_in[:, :, lo:hi])
        ident = sb.tile([P, P], BF16, name="ident")
        nc.sync.dma_start(out=ident, in_=ident_in)

        rsums = sb.tile([P, T_SLAB * NU + 3], F32, name="rsums")
        outb = sb.tile([P, 2 * T_SLAB], F32, name="outb")

        # the very first chunk is emitted as two 1024-wide halves so the
        # first exp only waits on the first two DMA pieces; its two partial
        # row-sums land in rsums cols 0 and 1 (all later chunks shift by +1)
        for u in range(NU):
            for m in range(T_SLAB):
                first = (u == 0 and m == 0)
                subw = CW // 4 if first else CW
                for s0 in range(CW // subw):
                    ps = psmm.tile([P, subw], F32, tag="ps", name="ps")
                    for h in range(subw // MMW):
                        off = u * CW + s0 * subw + h * MMW
                        nc.tensor.matmul(ps[:, h * MMW:(h + 1) * MMW],
                                         xT[:, :, m * P:(m + 1) * P],
                                         xT[:, :, off:off + MMW],
                                         start=True, stop=True, perf_mode=DR)
                    # psum holds 16*cos; exp(2*cos) = exp(psum * 0.125).
                    # Row-sums ride the DVE STT accumulator, except the last
                    # T_SLAB-1 chunks which use the ACT accumulator (the DVE
                    # backlog would otherwise trail the final exp).
                    ci = s0 if first else m * NU + u + 3
                    ri = rsums[:, ci:ci + 1]
                    esc = sb.tile([P, CW], BF16, tag="esc", name="esc",
                                  bufs=12)
                    if u == NU - 1 and m >= 1:
                        nc.scalar.activation(esc[:, :subw], ps, AF.Exp,
                                             scale=2.0 / (FP8_SCALE ** 2),
                                             accum_out=ri)
                    else:
                        nc.scalar.activation(esc[:, :subw], ps, AF.Exp,
                                             scale=2.0 / (FP8_SCALE ** 2))
                        esc2 = sb.tile([P, CW], BF16, tag="esc2",
                                       name="esc2", bufs=4)
                        nc.vector.scalar_tensor_tensor(
                            esc2[:, :subw], esc[:, :subw], 1.0,
                            esc[:, :subw], ALU.mult, ALU.max, accum_out=ri)
                if u == 2:
                    # positive diagonal (cols 4096+m*128..+128) from the exp'd
                    # chunk: host recovers 2*cos with a log
                    scr = sb.tile([P, P], BF16, tag="scr", name="scr", bufs=2)
                    nc.vector.tensor_mul(scr, esc[:, m * P:(m + 1) * P],
                                         ident)
                    nc.vector.reduce_sum(outb[:, T_SLAB + m:T_SLAB + m + 1],
                                         scr, axis=mybir.AxisListType.X)
                if u == NU - 1:
                    lo = 0 if m == 0 else m * NU + 3
                    nc.vector.reduce_sum(outb[:, m:m + 1],
                                         rsums[:, lo:(m + 1) * NU + 3],
                                         axis=mybir.AxisListType.X)
            if u == 2:
                # ship the positives half early; only the row-sum half
                # remains for the final transfer
                nc.sync.dma_start(out=out_ap[:, T_SLAB:],
                                  in_=outb[:, T_SLAB:])
        nc.sync.dma_start(out=out_ap[:, :T_SLAB], in_=outb[:, :T_SLAB])


def build_nc():
    nc = bacc.Bacc("TRN2", target_bir_lowering=False, debug=False,
                   num_devices=NCORES)
    xT_in = nc.dram_tensor("xT", [P, KC, N2], FP8, kind="ExternalInput").ap()
    ident_in = nc.dram_tensor("ident", [P, P], BF16,
                              kind="ExternalInput").ap()
    out = nc.dram_tensor("out", [P, 2 * T_SLAB], F32,
                         kind="ExternalOutput").ap()
    with tile.TileContext(nc) as tc:
        _build_kernel(tc, out, xT_in, ident_in)
    nc.compile()
    return nc


_NC = None
LAST_RESULTS = None
_IDENT = np.eye(P, dtype=np.float32).astype(ml_dtypes.bfloat16)


def _make_in_maps(rep: np.ndarray):
    norm = np.maximum(np.sqrt((rep.astype(np.float64) ** 2).sum(1,
                                                                keepdims=True)),
                      1e-8)
    xh8 = (rep * (FP8_SCALE / norm)).astype(ml_dtypes.float8_e4m3)
    in_maps = []
    for c in range(NCORES):
        rot = np.roll(xh8, -c * ROWS, axis=0)  # col j = global row j + c*1024
        # xT[d, k, j] = rot[j, k*128 + d]
        xT = np.ascontiguousarray(
            rot.reshape(N2, KC, P).transpose(2, 1, 0))
        in_maps.append({"xT": xT, "ident": _IDENT})
    return in_maps


def kernel(representation: np.ndarray, **run_kwargs) -> np.ndarray:
    global _NC, LAST_RESULTS
    rep = np.ascontiguousarray(np.asarray(representation), dtype=np.float32)
    assert rep.shape == (N2, D)
    if _NC is None:
        _NC = build_nc()
    res = run_bass_kernel_spmd(_NC, _make_in_maps(rep),
                               core_ids=list(range(NCORES)), **run_kwargs)
    LAST_RESULTS = res
    total = 0.0
    e2 = float(np.exp(2.0))
    for r in res.results:
        out = r["out"].astype(np.float64)
        S = out[:, :T_SLAB]
        pos_exp = out[:, T_SLAB:]          # = exp(2*cos) of the positive pair
        total += float((np.log(S - e2) - np.log(pos_exp)).sum())
    return np.asarray(np.float32(total / N2))


# revision 21
# speedup vs baseline: 1.0164x; 1.0164x over previous
"""NT-Xent (SimCLR) contrastive loss on 8 Trainium2 NeuronCores.

Math: with x_hat = row-normalized representation [8192, 256], tau = 0.5,
  sim = x_hat @ x_hat.T
  loss = (1/8192) * sum_i [ ln(sum_{j!=i} exp(2 sim[i,j])) - 2 sim[i, pos(i)] ]
where pos(i) = (i + 4096) mod 8192.

Sharding: data-parallel over rows; core c owns rows [c*1024, (c+1)*1024).
The host pre-normalizes rows, quantizes to fp8e4m3 (scaled by 4), and ships
each core a pre-transposed, row-rotated key matrix xT [128, 2, 8192] so a
single SPMD program works on every core: column j of core c's slab is global
row (j + c*1024) % 8192, which puts each core's own rows at columns 0..1023
(the matmul stationary tiles) and every core's positive diagonal at columns
4096..5119.

On device, per core: 128 fp8 DoubleRow matmuls (K=256 in one pass) build the
[1024, 8192] similarity slab in [128, 2048] PSUM chunks; the ACT engine does
exp (the scale folds the 1/16 fp8 scaling and 1/tau); row-sums ride the DVE
scalar_tensor_tensor accumulator except the last few chunks, which use the
ACT accumulator so the kernel doesn't end DVE-bound. The positive diagonal is
read lazily from the exp'd u=2 chunk (identity mask + reduce; the host takes
ln to recover 2*cos). Output is [128, 16] per core; the host finishes with
ln(S - e^2) - ln(pos_exp) summed over rows.
"""

import numpy as np
import ml_dtypes

import concourse.bacc as bacc
import concourse.bass as bass
import concourse.tile as tile
from concourse import mybir
from concourse.bass_utils import run_bass_kernel_spmd

N2 = 8192            # total rows (2N)
D = 256              # feature dim
NCORES = 8
ROWS = N2 // NCORES  # 1024 rows per core
N = N2 // 2          # positive-pair offset
P = 128              # SBUF partitions
KC = 2               # two 128-row contraction chunks (K=256 via DoubleRow)
T_SLAB = ROWS // P   # 8 row tiles of the slab
CW = 2048            # exp chunk width (4 PSUM banks)
NU = N2 // CW        # 4 chunks per slab row tile
MMW = 512            # matmul moving free width (1 PSUM bank)
FP8_SCALE = 4.0      # x_hat quantized as x_hat * 4 -> sim psum = 16*cos

F32 = mybir.dt.float32
BF16 = mybir.dt.bfloat16
FP8 = mybir.dt.float8e4
AF = mybir.ActivationFunctionType
ALU = mybir.AluOpType
DR = mybir.MatmulPerfMode.DoubleRow


def _build_kernel(tc: tile.TileContext, out_ap, xT_in, ident_in):
    nc = tc.nc
    with (
        tc.tile_pool(name="sb", bufs=1) as sb,
        tc.tile_pool(name="psmm", bufs=2, space="PSUM") as psmm,
    ):
        xT = sb.tile([P, KC, N2], FP8, name="xT")
        # stream the key matrix in column pieces so matmuls can start early;
        # the first piece is small to cut the pipeline-fill latency
        splits = [0, 512, 1024, 2048, 4096, N2]
        for lo, hi in zip(splits, splits[1:]):
            nc.sync.dma_start(out=xT[:, :, lo:hi], in_=xT_in[:, :, lo:hi])
        ident = sb.tile([P, P], BF16, name="ident")
        nc.sync.dma_start(out=ident, in_=ident_in)

        rsums = sb.tile([P, T_SLAB * NU + 1], F32, name="rsums")
        outb = sb.tile([P, 2 * T_SLAB], F32, name="outb")

        # the very first chunk is emitted as two 1024-wide halves so the
        # first exp only waits on the first two DMA pieces; its two partial
        # row-sums land in rsums cols 0 and 1 (all later chunks shift by +1)
        for u in range(NU):
            for m in range(T_SLAB):
                first = (u == 0 and m == 0)
                subw = CW // 2 if first else CW
                for s0 in range(CW // subw):
                    ps = psmm.tile([P, subw], F32, tag="ps", name="ps")
                    for h in range(subw // MMW):
                        off = u * CW + s0 * subw + h * MMW
                        nc.tensor.matmul(ps[:, h * MMW:(h + 1) * MMW],
                                         xT[:, :, m * P:(m + 1) * P],
                                         xT[:, :, off:off + MMW],
                                         start=True, stop=True, perf_mode=DR)
                    # psum holds 16*cos; exp(2*cos) = exp(psum * 0.125).
                    # Row-sums ride the DVE STT accumulator, except the last
                    # T_SLAB-1 chunks which use the ACT accumulator (the DVE
                    # backlog would otherwise trail the final exp).
                    ci = s0 if first else m * NU + u + 1
                    ri = rsums[:, ci:ci + 1]
                    esc = sb.tile([P, CW], BF16, tag="esc", name="esc",
                                  bufs=12)
                    if u == NU - 1 and m >= 1:
                        nc.scalar.activation(esc[:, :subw], ps, AF.Exp,
                                             scale=2.0 / (FP8_SCALE ** 2),
                                             accum_out=ri)
                    else:
                        nc.scalar.activation(esc[:, :subw], ps, AF.Exp,
                                             scale=2.0 / (FP8_SCALE ** 2))
                        esc2 = sb.tile([P, CW], BF16, tag="esc2",
                                       name="esc2", bufs=4)
                        nc.vector.scalar_tensor_tensor(
                            esc2[:, :subw], esc[:, :subw], 1.0,
                            esc[:, :subw], ALU.mult, ALU.max, accum_out=ri)
                if u == 2:
                    # positive diagonal (cols 4096+m*128..+128) from the exp'd
                    # chunk: host recovers 2*cos with a log
                    scr = sb.tile([P, P], BF16, tag="scr", name="scr", bufs=2)
                    nc.vector.tensor_mul(scr, esc[:, m * P:(m + 1) * P],
                                         ident)
                    nc.vector.reduce_sum(outb[:, T_SLAB + m:T_SLAB + m + 1],
                                         scr, axis=mybir.AxisListType.X)
                if u == NU - 1:
                    lo = 0 if m == 0 else m * NU + 1
                    nc.vector.reduce_sum(outb[:, m:m + 1],
                                         rsums[:, lo:(m + 1) * NU + 1],
                                         axis=mybir.AxisListType.X)
            if u == 2:
                # ship the positives half early; only the row-sum half
                # remains for the final transfer
                nc.sync.dma_start(out=out_ap[:, T_SLAB:],
                                  in_=outb[:, T_SLAB:])
        nc.sync.dma_start(out=out_ap[:, :T_SLAB], in_=outb[:, :T_SLAB])


def build_nc():
    nc = bacc.Bacc("TRN2", target_bir_lowering=False, debug=False,
                   num_devices=NCORES)
    xT_in = nc.dram_tensor("xT", [P, KC, N2], FP8, kind="ExternalInput").ap()
    ident_in = nc.dram_tensor("ident", [P, P], BF16,
                              kind="ExternalInput").ap()
    out = nc.dram_tensor("out", [P, 2 * T_SLAB], F32,
                         kind="ExternalOutput").ap()
    with tile.TileContext(nc) as tc:
        _build_kernel(tc, out, xT_in, ident_in)
    nc.compile()
    return nc


_NC = None
LAST_RESULTS = None
_IDENT = np.eye(P, dtype=np.float32).astype(ml_dtypes.bfloat16)


def _make_in_maps(rep: np.ndarray):
    norm = np.maximum(np.sqrt((rep.astype(np.float64) ** 2).sum(1,
                                                                keepdims=True)),
                      1e-8)
    xh8 = (rep * (FP8_SCALE / norm)).astype(ml_dtypes.float8_e4m3)
    in_maps = []
    for c in range(NCORES):
        rot = np.roll(xh8, -c * ROWS, axis=0)  # col j = global row j + c*1024
        # xT[d, k, j] = rot[j, k*128 + d]
        xT = np.ascontiguousarray(
            rot.reshape(N2, KC, P).transpose(2, 1, 0))
        in_maps.append({"xT": xT, "ident": _IDENT})
    return in_maps


def kernel(representation: np.ndarray, **run_kwargs) -> np.ndarray:
    global _NC, LAST_RESULTS
    rep = np.ascontiguousarray(np.asarray(representation), dtype=np.float32)
    assert rep.shape == (N2, D)
    if _NC is None:
        _NC = build_nc()
    res = run_bass_kernel_spmd(_NC, _make_in_maps(rep),
                               core_ids=list(range(NCORES)), **run_kwargs)
    LAST_RESULTS = res
    total = 0.0
    e2 = float(np.exp(2.0))
    for r in res.results:
        out = r["out"].astype(np.float64)
        S = out[:, :T_SLAB]
        pos_exp = out[:, T_SLAB:]          # = exp(2*cos) of the positive pair
        total += float((np.log(S - e2) - np.log(pos_exp)).sum())
    return np.asarray(np.float32(total / N2))
